# revision 1
# baseline (speedup 1.0000x reference)
"""LoRA SwiGLU MLP on 8 Trainium2 NeuronCores.

Strategy:
  - Fold LoRA into the dense weights on host: W' = W + B @ A (exact, fp32).
    The device then computes a plain SwiGLU MLP: out = silu(x@Wg'.T) * (x@Wu'.T) @ Wd'.T
  - Data-parallel over tokens: each of the 8 cores gets 512 of the 4096 tokens
    and the full (folded, bf16) weights. No collectives; host concatenates.
  - All matmuls keep the contraction dim on partitions with weights as the
    stationary operand and the 512-token activation tile as the moving operand:
      gate/up: psum[h128, m512] += WgT_tile[k128, h128].T @ xT[k128, m512]   (32 k-steps)
      down:    psum[d128, m512] += WdT_tile[h128, d128].T @ hT[h128, m512]   (86 h-steps)
  - silu+mul fused on ACT/DVE between phases; hT ([128, 86, 512] bf16) stays in SBUF.
"""

import numpy as np
import ml_dtypes

import concourse.mybir as mybir
import concourse.tile as tile
from concourse import bacc
from concourse.bass_utils import run_bass_kernel_spmd

P = 128
D_MODEL = 4096
D_HIDDEN = 11008
RANK = 16
BATCH, SEQ = 2, 2048
TOK = BATCH * SEQ          # 4096 tokens
N_CORES = 8
M = TOK // N_CORES         # 512 tokens per core
KT = D_MODEL // P          # 32 contraction tiles for gate/up
HT = D_HIDDEN // P         # 86 hidden tiles
DT = D_MODEL // P          # 32 output tiles for down

BF16 = mybir.dt.bfloat16
F32 = mybir.dt.float32
NP_BF16 = ml_dtypes.bfloat16

_NC_CACHE = {}


def _build_nc():
    nc = bacc.Bacc("TRN2")
    xt_d = nc.dram_tensor("xt", [P, KT, M], BF16, kind="ExternalInput")
    wg_d = nc.dram_tensor("wg", [HT, P, KT, P], BF16, kind="ExternalInput")
    wu_d = nc.dram_tensor("wu", [HT, P, KT, P], BF16, kind="ExternalInput")
    wd_d = nc.dram_tensor("wd", [DT, P, HT, P], BF16, kind="ExternalInput")
    ot_d = nc.dram_tensor("ot", [DT, P, M], F32, kind="ExternalOutput")

    with tile.TileContext(nc) as tc:
        with (
            tc.tile_pool(name="singles", bufs=1) as singles,
            tc.tile_pool(name="wgu", bufs=2) as wgu,
            tc.tile_pool(name="wdp", bufs=2) as wdp,
            tc.tile_pool(name="tmp", bufs=2) as tmpp,
            tc.tile_pool(name="ostg", bufs=4) as ostg,
            tc.tile_pool(name="pgu", bufs=2, space="PSUM") as pgu,
            tc.tile_pool(name="pdp", bufs=2, space="PSUM") as pdp,
        ):
            xt = singles.tile([P, KT, M], BF16)
            hT = singles.tile([P, HT, M], BF16)
            nc.sync.dma_start(out=xt, in_=xt_d[:])

            # ---- gate/up + silu*mul ----
            for h in range(HT):
                wg_t = wgu.tile([P, KT, P], BF16, tag="wg")
                wu_t = wgu.tile([P, KT, P], BF16, tag="wu")
                nc.sync.dma_start(out=wg_t, in_=wg_d[h])
                nc.sync.dma_start(out=wu_t, in_=wu_d[h])
                pg = pgu.tile([P, M], F32, tag="pg")
                pu = pgu.tile([P, M], F32, tag="pu")
                for k in range(KT):
                    nc.tensor.matmul(pg, wg_t[:, k, :], xt[:, k, :],
                                     start=(k == 0), stop=(k == KT - 1))
                for k in range(KT):
                    nc.tensor.matmul(pu, wu_t[:, k, :], xt[:, k, :],
                                     start=(k == 0), stop=(k == KT - 1))
                sg = tmpp.tile([P, M], F32, tag="sg")
                nc.scalar.activation(sg, pg, mybir.ActivationFunctionType.Silu)
                nc.vector.tensor_mul(out=hT[:, h, :], in0=sg, in1=pu)

            # ---- down ----
            for d in range(DT):
                wd_t = wdp.tile([P, HT, P], BF16, tag="wd")
                nc.sync.dma_start(out=wd_t, in_=wd_d[d])
                pd = pdp.tile([P, M], F32, tag="pd")
                for kh in range(HT):
                    nc.tensor.matmul(pd, wd_t[:, kh, :], hT[:, kh, :],
                                     start=(kh == 0), stop=(kh == HT - 1))
                o = ostg.tile([P, M], F32, tag="o")
                nc.vector.tensor_copy(out=o, in_=pd)
                nc.sync.dma_start(out=ot_d[d], in_=o)

    nc.finalize()
    return nc


def _get_nc():
    if "nc" not in _NC_CACHE:
        _NC_CACHE["nc"] = _build_nc()
    return _NC_CACHE["nc"]


def _prepare_inputs(x, gate_w, up_w, down_w, gate_a, gate_b, up_a, up_b,
                    down_a, down_b):
    f = np.float32
    x = np.asarray(x, f).reshape(TOK, D_MODEL)
    wg = np.asarray(gate_w, f) + np.asarray(gate_b, f) @ np.asarray(gate_a, f)
    wu = np.asarray(up_w, f) + np.asarray(up_b, f) @ np.asarray(up_a, f)
    wd = np.asarray(down_w, f) + np.asarray(down_b, f) @ np.asarray(down_a, f)

    # wg_dev[h, p, k, c] = wg[h*128+c, k*128+p]
    wg_dev = np.ascontiguousarray(
        wg.reshape(HT, P, KT, P).transpose(0, 3, 2, 1)).astype(NP_BF16)
    wu_dev = np.ascontiguousarray(
        wu.reshape(HT, P, KT, P).transpose(0, 3, 2, 1)).astype(NP_BF16)
    # wd_dev[d, p, kh, c] = wd[d*128+c, kh*128+p]
    wd_dev = np.ascontiguousarray(
        wd.reshape(DT, P, HT, P).transpose(0, 3, 2, 1)).astype(NP_BF16)
    # x_dev[core, p, k, m] = x[core*512+m, k*128+p]
    x_dev = np.ascontiguousarray(
        x.reshape(N_CORES, M, KT, P).transpose(0, 3, 2, 1)).astype(NP_BF16)

    in_maps = [
        {"xt": x_dev[c], "wg": wg_dev, "wu": wu_dev, "wd": wd_dev}
        for c in range(N_CORES)
    ]
    return in_maps


def _assemble(results):
    out = np.empty((TOK, D_MODEL), np.float32)
    for c in range(N_CORES):
        oc = results[c]["ot"].reshape(D_MODEL, M)  # [d, m]
        out[c * M:(c + 1) * M, :] = oc.T
    return out.reshape(BATCH, SEQ, D_MODEL)


def run(trace=False, **inputs):
    nc = _get_nc()
    in_maps = _prepare_inputs(**inputs)
    res = run_bass_kernel_spmd(nc, in_maps, core_ids=list(range(N_CORES)),
                               trace=trace)
    return _assemble(res.results), res


def kernel(**inputs):
    out, _ = run(trace=False, **inputs)
    return out


# revision 5
# speedup vs baseline: 1.4678x; 1.4678x over previous
"""LoRA SwiGLU MLP on 8 Trainium2 NeuronCores.

Strategy:
  - Fold LoRA into the dense weights on host: W' = W + B @ A (exact, fp32).
    The device then computes a plain SwiGLU MLP: out = silu(x@Wg'.T) * (x@Wu'.T) @ Wd'.T
  - Data-parallel over tokens: each of the 8 cores gets 512 of the 4096 tokens
    and the full (folded, bf16) weights. No collectives; host concatenates.
  - All matmuls keep the contraction dim on partitions with weights as the
    stationary operand and the 512-token activation tile as the moving operand:
      gate/up: psum[h128, m512] += WgT_tile[k128, h128].T @ xT[k128, m512]   (32 k-steps)
      down:    psum[d128, m512] += WdT_tile[h128, d128].T @ hT[h128, m512]   (86 h-steps)
  - silu+mul fused on ACT/DVE between phases; hT ([128, 86, 512] bf16) stays in SBUF.
"""

import numpy as np
import ml_dtypes

import concourse.mybir as mybir
import concourse.tile as tile
from concourse import bacc
from concourse.bass_utils import run_bass_kernel_spmd

P = 128
D_MODEL = 4096
D_HIDDEN = 11008
RANK = 16
BATCH, SEQ = 2, 2048
TOK = BATCH * SEQ          # 4096 tokens
N_CORES = 8
M = TOK // N_CORES         # 512 tokens per core
KT = D_MODEL // P          # 32 contraction tiles for gate/up
HT = D_HIDDEN // P         # 86 hidden tiles
DT = D_MODEL // P          # 32 output tiles for down

BF16 = mybir.dt.float16
F32 = mybir.dt.float32
NP_BF16 = np.float16

_NC_CACHE = {}


def _build_nc(repeats=1):
    nc = bacc.Bacc("TRN2")
    xt_d = nc.dram_tensor("xt", [P, KT, M], BF16, kind="ExternalInput")
    wg_d = nc.dram_tensor("wg", [HT, P, KT, P], BF16, kind="ExternalInput")
    wu_d = nc.dram_tensor("wu", [HT, P, KT, P], BF16, kind="ExternalInput")
    wd_d = nc.dram_tensor("wd", [DT, P, HT, P], BF16, kind="ExternalInput")
    ot_d = nc.dram_tensor("ot", [DT, P, M], F32, kind="ExternalOutput")

    with tile.TileContext(nc) as tc:
        with (
            tc.tile_pool(name="singles", bufs=1) as singles,
            tc.tile_pool(name="wgu", bufs=2) as wgu,
            tc.tile_pool(name="wdp", bufs=2) as wdp,
            tc.tile_pool(name="tmp", bufs=2) as tmpp,
            tc.tile_pool(name="ostg", bufs=4) as ostg,
            tc.tile_pool(name="pgu", bufs=2, space="PSUM") as pgu,
            tc.tile_pool(name="pdp", bufs=2, space="PSUM") as pdp,
        ):
            xt = singles.tile([P, KT, M], BF16)
            hT = singles.tile([P, HT, M], BF16)
            nc.sync.dma_start(out=xt, in_=xt_d[:])

            # repeats>1 is only used by the timing harness (slope method)
            for _rep in range(repeats):
                # ---- gate/up + silu*mul ----
                for h in range(HT):
                    wg_t = wgu.tile([P, KT, P], BF16, tag="wg")
                    wu_t = wgu.tile([P, KT, P], BF16, tag="wu")
                    nc.sync.dma_start(out=wg_t, in_=wg_d[h])
                    nc.sync.dma_start(out=wu_t, in_=wu_d[h])
                    pg = pgu.tile([P, M], F32, tag="pg")
                    pu = pgu.tile([P, M], F32, tag="pu")
                    for k in range(KT):
                        nc.tensor.matmul(pg, wg_t[:, k, :], xt[:, k, :],
                                         start=(k == 0), stop=(k == KT - 1))
                    for k in range(KT):
                        nc.tensor.matmul(pu, wu_t[:, k, :], xt[:, k, :],
                                         start=(k == 0), stop=(k == KT - 1))
                    sg = tmpp.tile([P, M], F32, tag="sg")
                    nc.scalar.activation(sg, pg,
                                         mybir.ActivationFunctionType.Silu)
                    nc.vector.tensor_mul(out=hT[:, h, :], in0=sg, in1=pu)

                # ---- down ----
                for d in range(DT):
                    wd_t = wdp.tile([P, HT, P], BF16, tag="wd")
                    nc.sync.dma_start(out=wd_t, in_=wd_d[d])
                    pd = pdp.tile([P, M], F32, tag="pd")
                    for kh in range(HT):
                        nc.tensor.matmul(pd, wd_t[:, kh, :], hT[:, kh, :],
                                         start=(kh == 0), stop=(kh == HT - 1))
                    o = ostg.tile([P, M], F32, tag="o")
                    nc.vector.tensor_copy(out=o, in_=pd)
                    nc.sync.dma_start(out=ot_d[d], in_=o)

    nc.finalize()
    return nc


def _get_nc():
    if "nc" not in _NC_CACHE:
        _NC_CACHE["nc"] = _build_nc()
    return _NC_CACHE["nc"]


def _prepare_inputs(x, gate_w, up_w, down_w, gate_a, gate_b, up_a, up_b,
                    down_a, down_b):
    f = np.float32
    x = np.asarray(x, f).reshape(TOK, D_MODEL)
    wg = np.asarray(gate_w, f) + np.asarray(gate_b, f) @ np.asarray(gate_a, f)
    wu = np.asarray(up_w, f) + np.asarray(up_b, f) @ np.asarray(up_a, f)
    wd = np.asarray(down_w, f) + np.asarray(down_b, f) @ np.asarray(down_a, f)

    # wg_dev[h, p, k, c] = wg[h*128+c, k*128+p]
    wg_dev = np.ascontiguousarray(
        wg.reshape(HT, P, KT, P).transpose(0, 3, 2, 1)).astype(NP_BF16)
    wu_dev = np.ascontiguousarray(
        wu.reshape(HT, P, KT, P).transpose(0, 3, 2, 1)).astype(NP_BF16)
    # wd_dev[d, p, kh, c] = wd[d*128+c, kh*128+p]
    wd_dev = np.ascontiguousarray(
        wd.reshape(DT, P, HT, P).transpose(0, 3, 2, 1)).astype(NP_BF16)
    # x_dev[core, p, k, m] = x[core*512+m, k*128+p]
    x_dev = np.ascontiguousarray(
        x.reshape(N_CORES, M, KT, P).transpose(0, 3, 2, 1)).astype(NP_BF16)

    in_maps = [
        {"xt": x_dev[c], "wg": wg_dev, "wu": wu_dev, "wd": wd_dev}
        for c in range(N_CORES)
    ]
    return in_maps


def _assemble(results):
    out = np.empty((TOK, D_MODEL), np.float32)
    for c in range(N_CORES):
        oc = results[c]["ot"].reshape(D_MODEL, M)  # [d, m]
        out[c * M:(c + 1) * M, :] = oc.T
    return out.reshape(BATCH, SEQ, D_MODEL)


def run(trace=False, **inputs):
    nc = _get_nc()
    in_maps = _prepare_inputs(**inputs)
    res = run_bass_kernel_spmd(nc, in_maps, core_ids=list(range(N_CORES)),
                               trace=trace)
    return _assemble(res.results), res


def kernel(**inputs):
    out, _ = run(trace=False, **inputs)
    return out
